# revision 47
# baseline (speedup 1.0000x reference)
"""BaiChuan attention layer on 8 Trainium2 NeuronCores.

Sharding: tensor-parallel over heads across all 8 cores (4 heads per
core, both batches on every core).  o_proj is token-parallel: per-head
attention outputs are exchanged with one 8-way AllToAll per causal
token-half, after which each core computes the full o_proj for its
(batch, token-quarter) share with w_o streamed from HBM.

Precision: q/k projection runs in fp8e4 with DoubleRow (inputs scaled by
64, descaled via the rope cos/sin tables which are pre-divided by 4096);
everything else is bf16 operands with f32 PSUM accumulation.  Validated
end-to-end in numpy at rel err 3.4e-3 (budget 2e-2).

Pipeline (core c: heads 4c..4c+4 of both batches; owns output tokens
[256*(c%4), +256) and [1024+256*(c%4), +256) of batch c//4):
  stage A  (8 token blocks of 512 over batch0++batch1): qkvT[j,t];
           v in bf16, q/k in fp8 DoubleRow.
  pass 1   (after blocks 0,1 / 4,5): per head-instance: RoPE q,k on
           tokens [0,1024), v transposes, causal attention for q in
           [0,1024) (final under causality), exp batched 4 tiles on
           ACT, denominator via full-partition ones matmul + fast
           reciprocal.
  A2A #1   8-way AllToAll of all instances' attn[:, 0:1024] (overlaps
           stage A blocks 2,3/6,7 and pass 2).
  pass 2   attention for q in [1024,2048) (k re-roped over full S).
  A2A #2   second half exchange.
  stage C  o_proj per token quarter: out[t, m] over all 32 heads,
           w_o streamed per 512-wide m block, split by causal half so
           half 1 overlaps pass 2.
"""
import sys
sys.path.insert(0, '/opt/trn_rl_repo')
import numpy as np
import ml_dtypes

import concourse.bass as bass
from concourse import bacc
import concourse.mybir as mybir
from concourse.tile import TileContext
from concourse.bass_utils import run_bass_kernel_spmd
from concourse.masks import make_identity

f32 = mybir.dt.float32
bf16 = mybir.dt.bfloat16
fp8 = mybir.dt.float8e4
AF = mybir.ActivationFunctionType
DR = mybir.MatmulPerfMode.DoubleRowSwInterleave

B, S, H, NH = 2, 2048, 4096, 32
HD = H // NH                    # 128
THETA = 10000.0
NCORES = 8
HPC = NH // NCORES              # 4 heads per core
NI = B * HPC                    # 8 head-instances (batch, head) per core
JCC = HPC * HD                  # 512 = per-core head width (one batch)
SCALE = HD ** -0.5
SCALEQK = 64.0                  # fp8 input scaling for the q/k projection
GROUPS = [[0, 1, 2, 3, 4, 5, 6, 7]]
TB = 512                        # stage-A token block
NTB = B * S // TB               # 8 blocks over batch0 ++ batch1
NIB = H // 128                  # 32 contraction blocks
TH = S // 2                     # causal token half
QT = 256                        # tokens per (core, half)
NIB2 = NIB // 2
NJB = NH                        # 32 o_proj j-blocks


def build_nc():
    nc = bacc.Bacc(None)
    hsT16 = nc.declare_dram_parameter("hsT16", [H, B * S], bf16,
                                      isOutput=False)
    hsT8 = nc.declare_dram_parameter("hsT8", [H, B * S], fp8, isOutput=False)
    # SwInterleave stationary blocks: [128 p, ob*16+ibpair, 256 interleaved]
    # ob = kq out-block (hh*2 + part) of this core, 8 per core
    wT8 = nc.declare_dram_parameter("wT8", [128, 8 * NIB2, 256], fp8,
                                    isOutput=False)
    wTv = nc.declare_dram_parameter("wTv", [H, JCC], bf16, isOutput=False)
    woT = nc.declare_dram_parameter("woT", [H, H], bf16, isOutput=False)
    cosf = nc.declare_dram_parameter("cosf", [HD, S], bf16, isOutput=False)
    sinm = nc.declare_dram_parameter("sinm", [HD, S], bf16, isOutput=False)
    masks = nc.declare_dram_parameter("masks", [4, 128, 512], bf16,
                                      isOutput=False)
    out = nc.declare_dram_parameter("out", [2 * QT, H], f32, isOutput=True)

    # per head-instance i = b*HPC + hh: k/q at kq_d[2i + 0/1]
    kq_d = [nc.dram_tensor(f"kq_d{j}", [128, S], bf16)
            for j in range(2 * NI)]
    # v in natural [t, j] layout, one tensor per batch
    v_d = [nc.dram_tensor(f"v_d{b}", [S, JCC], bf16) for b in range(B)]
    a2a_in = [nc.dram_tensor(f"a2a{i}_in", [NCORES, JCC, QT], bf16)
              for i in range(2)]
    a2a_out = [nc.dram_tensor(f"a2a{i}_out", [NCORES, JCC, QT], bf16)
               for i in range(2)]

    hsT16_v = hsT16[:].rearrange("(n p) t -> p n t", p=128)   # [128, 32, BS]
    hsT8_v = hsT8[:].rearrange("(n p) t -> p n t", p=128)
    wTv_v = wTv[:].rearrange("(n p) j -> p n j", p=128)       # [128, 32, 512]
    v_d_v = [t[:].rearrange("(kb p) j -> p kb j", p=128)      # [128, 16, 512]
             for t in v_d]
    woT_v = woT[:].rearrange("(n p) m -> p n m", p=128)       # [128, 32, H]
    at_v = [t[:].rearrange("r (n p) t -> p (r n) t", p=128)   # [128, 32, QT]
            for t in a2a_out]

    with TileContext(nc) as tc:
        with tc.tile_pool(name="const", bufs=1) as pconst, \
             tc.tile_pool(name="stA", bufs=1) as pa, \
             tc.tile_pool(name="stB", bufs=1) as pb, \
             tc.tile_pool(name="stC", bufs=1) as pc, \
             tc.tile_pool(name="psum", bufs=1, space="PSUM") as ps:
            # ---------------- constants (PE warmup first) ----------------
            ident = pconst.tile([128, 128], bf16, tag="ident", bufs=1)
            make_identity(nc, ident[:])
            ones8 = pconst.tile([128, 256], fp8, tag="ones8", bufs=1)
            nc.vector.memset(ones8[:], 1.0)
            # PE warmup burst (HAM un-throttle while first DMAs land)
            for wu in range(48):
                pwu = ps.tile([128, 128], f32, tag="po", bufs=1,
                              name=f"warm_{wu}")
                nc.tensor.matmul(pwu[:], ident[:], ident[:],
                                 start=True, stop=True)
            cos_sb = pconst.tile([128, S], bf16, tag="cos", bufs=1)
            sin_sb = pconst.tile([128, S], bf16, tag="sin", bufs=1)
            mask_sb = pconst.tile([128, 4, 512], bf16, tag="mask", bufs=1)

            def load_consts():   # emitted after stage A block 0's DMAs
                nc.sync.dma_start(out=cos_sb[:], in_=cosf[:])
                nc.sync.dma_start(out=sin_sb[:], in_=sinm[:])
                nc.sync.dma_start(out=mask_sb[:],
                                  in_=masks[:].rearrange("v p x -> p v x"))
                # zero the score slot once so narrowed diagonal tiles never
                # exp() uninitialized PSUM (later reuses see old scores)
                pss0 = ps.tile([128, 4, 512], f32, tag="pss", bufs=1,
                               name="pss_init")
                nc.vector.memset(pss0[:], 0.0)

            # ---------------- stage A: fused QKV projection ----------------
            def stage_a(tb, kq_first=False):
                u0 = tb * TB                  # global token (batch-major)
                b = u0 // S
                t0 = u0 % S
                if kq_first:      # block 0: smallest possible first wait
                    hs8 = pa.tile([128, NIB, TB], fp8, tag="hs8", bufs=1,
                                  name=f"hs8_{tb}")
                    nc.sync.dma_start(out=hs8[:],
                                      in_=hsT8_v[:, :, u0:u0 + TB])
                hs16a = pa.tile([128, NIB2, TB], bf16, tag="hs16a", bufs=1,
                                name=f"hs16a_{tb}")
                hs16b = pa.tile([128, NIB2, TB], bf16, tag="hs16b", bufs=1,
                                name=f"hs16b_{tb}")
                nc.sync.dma_start(out=hs16a[:],
                                  in_=hsT16_v[:, :NIB2, u0:u0 + TB])
                nc.sync.dma_start(out=hs16b[:],
                                  in_=hsT16_v[:, NIB2:, u0:u0 + TB])
                if not kq_first:
                    hs8 = pa.tile([128, NIB, TB], fp8, tag="hs8", bufs=1,
                                  name=f"hs8_{tb}")
                    nc.sync.dma_start(out=hs8[:],
                                      in_=hsT8_v[:, :, u0:u0 + TB])
                # v phase is hs-stationary into natural [t, j] layout; with
                # v first, hs16 dies early so the next block's hs16 loads
                # overlap the kq phase.  Block 0 runs kq first (smaller
                # first-DMA wait).
                def v_phase():
                    wva = pa.tile([128, NIB2, JCC], bf16, tag="wva", bufs=1,
                                  name=f"wva_{tb}")
                    wvb = pa.tile([128, NIB2, JCC], bf16, tag="wvb", bufs=1,
                                  name=f"wvb_{tb}")
                    nc.sync.dma_start(out=wva[:], in_=wTv_v[:, :NIB2, :])
                    nc.sync.dma_start(out=wvb[:], in_=wTv_v[:, NIB2:, :])
                    for tt in range(TB // 128):
                        psa = ps.tile([128, JCC], f32, tag="psA", bufs=2,
                                      name=f"psA_v_{tb}_{tt}")
                        for ib in range(NIB):
                            hsrc = hs16a if ib < NIB2 else hs16b
                            wsrc = wva if ib < NIB2 else wvb
                            nc.tensor.matmul(
                                psa[:],
                                hsrc[:, ib % NIB2, tt * 128:(tt + 1) * 128],
                                wsrc[:, ib % NIB2, :],
                                start=(ib == 0), stop=(ib == NIB - 1))
                        st = pa.tile([128, JCC], bf16, tag="oA", bufs=2,
                                     name=f"stA_{tb}_v_{tt}")
                        if tt % 2 == 0:
                            nc.scalar.copy(st[:], psa[:])
                        else:
                            nc.vector.tensor_copy(st[:], psa[:])
                        nc.sync.dma_start(
                            out=v_d[b][:][t0 + tt * 128:
                                          t0 + (tt + 1) * 128, :],
                            in_=st[:])

                def kq_phase():
                    for hp in range(HPC // 2):
                        w8 = pa.tile([128, 4 * NIB2, 256], fp8, tag="w8",
                                     bufs=2, name=f"w8_{tb}_{hp}")
                        nc.sync.dma_start(
                            out=w8[:],
                            in_=wT8[:][:, hp * 4 * NIB2:
                                       (hp + 1) * 4 * NIB2, :])
                        for d in range(2):
                            hh = 2 * hp + d
                            i = b * HPC + hh
                            for part in range(2):
                                lob = 2 * d + part
                                psb = ps.tile([128, TB], f32, tag="psA",
                                              bufs=2,
                                              name=f"psA_kq_{tb}_{hh}_{part}")
                                for ii in range(NIB2):
                                    nc.tensor.matmul(
                                        psb[:],
                                        w8[:, lob * NIB2 + ii, :],
                                        hs8[:, 2 * ii:2 * ii + 2, :],
                                        start=(ii == 0),
                                        stop=(ii == NIB2 - 1),
                                        perf_mode=DR)
                                st2 = pa.tile([128, TB], bf16, tag="oA",
                                              bufs=2,
                                              name=f"stA_{tb}_{hh}_{part}")
                                if part == 0:
                                    nc.vector.tensor_copy(st2[:], psb[:])
                                else:
                                    nc.scalar.copy(st2[:], psb[:])
                                nc.sync.dma_start(
                                    out=kq_d[2 * i + part][:][:, t0:t0 + TB],
                                    in_=st2[:])

                if kq_first:
                    kq_phase()
                    v_phase()
                else:
                    v_phase()
                    kq_phase()

            # ------------- stage B helpers -------------
            def load_rope(jt, c0, c1, tag, nm):
                X = c1 - c0
                raw = pb.tile([128, X], bf16, tag="raw", bufs=5,
                              name=f"{nm}_raw")
                nc.sync.dma_start(out=raw[:], in_=kq_d[jt][:][:, c0:c1])
                sw = pb.tile([128, X], bf16, tag="raw", bufs=5,
                             name=f"{nm}_sw")
                nc.sync.dma_start(out=sw[0:64, :],
                                  in_=kq_d[jt][:][64:128, c0:c1])
                nc.sync.dma_start(out=sw[64:128, :],
                                  in_=kq_d[jt][:][0:64, c0:c1])
                t2 = pb.tile([128, X], bf16, tag="ropetmp", bufs=2,
                             name=f"{nm}_t2")
                rt = pb.tile([128, X], bf16, tag=tag, bufs=2,
                             name=f"{nm}_roped")
                with tc.high_priority():
                    nc.vector.tensor_mul(t2[:], sw[:], sin_sb[:, c0:c1])
                    nc.vector.tensor_mul(rt[:], raw[:], cos_sb[:, c0:c1])
                    nc.vector.tensor_add(rt[:], rt[:], t2[:])
                return rt

            def attn_block(i, g, kT, qT, qoff, v_sb, a2a_t, half):
                """causal attention for q block g (512 q), k blocks 0..4g+3"""
                b, hh = divmod(i, HPC)
                nbat = g + 1
                po = ps.tile([128, 512], f32, tag="po", bufs=1,
                             name=f"po_{half}_{i}_{g}")
                pden = ps.tile([128, 512], f32, tag="pden", bufs=1,
                               name=f"pden_{half}_{i}_{g}")
                for bt in range(nbat):
                    diag = (bt == nbat - 1)
                    pss = ps.tile([128, 4, 512], f32, tag="pss", bufs=1,
                                  name=f"pss_{half}_{i}_{g}_{bt}")
                    for j in range(4):
                        kb = 4 * bt + j
                        off = 128 * j if diag else 0
                        nc.tensor.matmul(
                            pss[:, j, off:512],
                            kT[:, kb * 128:(kb + 1) * 128],
                            qT[:, qoff + off:qoff + 512],
                            start=True, stop=True)
                    pt = pb.tile([128, 4, 512], bf16, tag="pt", bufs=2,
                                 name=f"pt_{half}_{i}_{g}_{bt}")
                    nc.scalar.activation(pt[:], pss[:], AF.Exp, scale=SCALE)
                    if diag:
                        nc.vector.tensor_mul(pt[:], pt[:], mask_sb[:])
                    pt8 = pb.tile([128, 4, 512], fp8, tag="pt8", bufs=2,
                                  name=f"pt8_{half}_{i}_{g}_{bt}")
                    nc.vector.tensor_copy(pt8[:], pt[:])
                    for jp in range(2):      # denominator: fp8 DoubleRow pairs
                        off = 256 * jp if diag else 0
                        nc.tensor.matmul(
                            pden[:, off:512], ones8[:],
                            pt8[:, 2 * jp:2 * jp + 2, off:512],
                            start=(bt == 0 and jp == 0),
                            stop=(bt == nbat - 1 and jp == 1),
                            perf_mode=DR)
                    for j in range(4):
                        kb = 4 * bt + j
                        off = 128 * j if diag else 0
                        nc.tensor.matmul(po[:, off:512], v_sb[:, kb, :],
                                         pt[:, j, off:512],
                                         start=(kb == 0),
                                         stop=(kb == 4 * nbat - 1))
                rden = pb.tile([128, 512], f32, tag="rden", bufs=1,
                               name=f"rden_{half}_{i}_{g}")
                nc.vector.reciprocal_approx_fast(out=rden[:], in_=pden[:])
                attn = pb.tile([128, 512], bf16, tag="attn", bufs=2,
                               name=f"attn_{half}_{i}_{g}")
                nc.vector.tensor_mul(attn[:], po[:], rden[:])
                gl = g - 2 * half      # quarter-pair index within the half
                for dq in range(2):
                    shard = b * (NCORES // B) + 2 * gl + dq
                    nc.sync.dma_start(
                        out=a2a_t[:][shard, hh * 128:(hh + 1) * 128, :],
                        in_=attn[:, dq * QT:(dq + 1) * QT])

            def load_v(i, nkb, half):
                b, hh = divmod(i, HPC)
                v_sb = pb.tile([128, nkb, 128], bf16, tag="vsb", bufs=2,
                               name=f"v_{half}_{i}")
                nc.sync.dma_start(
                    out=v_sb[:],
                    in_=v_d_v[b][:, 0:nkb, hh * 128:(hh + 1) * 128])
                return v_sb

            # ---------------- emit ----------------
            # stage A blocks: batch0 tokens [0,1024), batch1 [0,1024)
            def pass1_inst(i):
                kT = load_rope(2 * i, 0, TH, "kr_r", f"k1_{i}")
                qT = load_rope(2 * i + 1, 0, TH, "qr_r", f"q1_{i}")
                v_sb = load_v(i, TH // 128, 0)
                for g in range(2):
                    attn_block(i, g, kT, qT, g * 512, v_sb, a2a_in[0], 0)

            def pass2_inst(i):
                kT = load_rope(2 * i, 0, S, "kr_r", f"k2_{i}")
                qT = load_rope(2 * i + 1, TH, S, "qr_r", f"q2_{i}")
                v_sb = load_v(i, S // 128, 1)
                for g in range(2, 4):
                    attn_block(i, g, kT, qT, (g - 2) * 512, v_sb,
                               a2a_in[1], 1)

            with nc.named_scope("stageA01"):
                stage_a(0, kq_first=True)
                load_consts()
                stage_a(1)
            with nc.named_scope("pass1"):
                for i in range(HPC):           # batch-0 instances
                    pass1_inst(i)
            with nc.named_scope("stageA01b"):
                stage_a(4)
                stage_a(5)
            with nc.named_scope("pass1b"):
                for i in range(HPC, NI):       # batch-1 instances
                    pass1_inst(i)
            nc.gpsimd.collective_compute(
                "AllToAll", mybir.AluOpType.bypass, replica_groups=GROUPS,
                ins=[a2a_in[0][:]], outs=[a2a_out[0][:]])

            with nc.named_scope("stageA23"):
                stage_a(2)
                stage_a(3)
            with nc.named_scope("pass2"):
                for i in range(HPC):
                    pass2_inst(i)
            with nc.named_scope("stageA23b"):
                stage_a(6)
                stage_a(7)
            with nc.named_scope("pass2b"):
                for i in range(HPC, NI):
                    pass2_inst(i)
            nc.gpsimd.collective_compute(
                "AllToAll", mybir.AluOpType.bypass, replica_groups=GROUPS,
                ins=[a2a_in[1][:]], outs=[a2a_out[1][:]])

            # ---------------- stage C: token-quarter o_proj ----------------
            with nc.named_scope("stageC"):
                at_tags = ["hs8", "hs16a"]
                for half in range(2):
                    at = pa.tile([128, NJB, QT], bf16, tag=at_tags[half],
                                 bufs=1, name=f"at_{half}")
                    nc.scalar.dma_start(out=at[:], in_=at_v[half][:, :, :])
                    for mb in range(H // 512):
                        woA = pa.tile([128, 16, 512], bf16, tag="w8", bufs=2,
                                      name=f"woA_{half}_{mb}")
                        woB = pa.tile([128, 16, 512], bf16, tag="wvb", bufs=1,
                                      name=f"woB_{half}_{mb}")
                        nc.scalar.dma_start(
                            out=woA[:],
                            in_=woT_v[:, 0:16, mb * 512:(mb + 1) * 512])
                        nc.scalar.dma_start(
                            out=woB[:],
                            in_=woT_v[:, 16:32, mb * 512:(mb + 1) * 512])
                        for t in range(QT // 128):
                            psc = ps.tile([128, 512], f32, tag="psA", bufs=2,
                                          name=f"psC_{half}_{mb}_{t}")
                            for jb in range(NJB):
                                wsrc = woA if jb < 16 else woB
                                nc.tensor.matmul(
                                    psc[:],
                                    at[:, jb, t * 128:(t + 1) * 128],
                                    wsrc[:, jb % 16, :],
                                    start=(jb == 0), stop=(jb == NJB - 1))
                            oc = pc.tile([128, 512], f32, tag="oC", bufs=2,
                                         name=f"oC_{half}_{mb}_{t}")
                            nc.scalar.copy(oc[:], psc[:])
                            nc.sync.dma_start(
                                out=out[:][half * QT + t * 128:
                                           half * QT + (t + 1) * 128,
                                           mb * 512:(mb + 1) * 512],
                                in_=oc[:])

    nc.finalize()
    return nc


_NC_CACHE = None


def _get_nc():
    global _NC_CACHE
    if _NC_CACHE is None:
        _NC_CACHE = build_nc()
    return _NC_CACHE


def _host_inputs(hidden_states, positions, w_pack, w_o):
    hidden_states = np.asarray(hidden_states, dtype=np.float32)
    positions = np.asarray(positions)
    w_pack = np.asarray(w_pack, dtype=np.float32)
    w_o = np.asarray(w_o, dtype=np.float32)

    half = HD // 2
    inv_freq = (1.0 / (THETA ** (np.arange(half, dtype=np.float32) / half)))

    # causal mask variants for the 4 diagonal (128x512) tiles of a q-block
    masks = np.empty((4, 128, 512), dtype=np.float32)
    xs = np.arange(512)[None, :]
    ps = np.arange(128)[:, None]
    for v in range(4):
        masks[v] = (xs >= ps + 128 * v).astype(np.float32)

    woT_full = np.ascontiguousarray(w_o.T).astype(ml_dtypes.bfloat16)
    # both batches side by side: [H, B*S]
    hsT = np.concatenate([hidden_states[0].T, hidden_states[1].T], axis=1)
    hsT = np.ascontiguousarray(hsT)
    hsT16 = hsT.astype(ml_dtypes.bfloat16)
    hsT8 = (hsT * SCALEQK).astype(ml_dtypes.float8_e4m3)

    ang = positions[0].astype(np.float32)[None, :] * inv_freq[:, None]
    cos_t = np.cos(ang).astype(np.float32)                 # [64, S]
    sin_t = np.sin(ang).astype(np.float32)
    dsc = 1.0 / (SCALEQK * SCALEQK)
    cosf = (np.concatenate([cos_t, cos_t], axis=0) * dsc) \
        .astype(ml_dtypes.bfloat16)
    sinm = (np.concatenate([-sin_t, sin_t], axis=0) * dsc) \
        .astype(ml_dtypes.bfloat16)
    masks16 = masks.astype(ml_dtypes.bfloat16)

    in_maps = []
    for c in range(NCORES):
        heads = np.arange(HPC * c, HPC * (c + 1))
        kq_parts, v_parts = [], []
        for h in heads:
            hr = np.arange(h * HD, (h + 1) * HD)
            kq_parts += [w_pack[H + hr], w_pack[hr]]       # k then q
            v_parts.append(w_pack[2 * H + hr])
        wT_kq = np.concatenate(kq_parts, axis=0).T             # [H, 2*JCC]
        # SwInterleave stationary: [p, ob*16+i, 256] with col 2c+m =
        # member m's weight column (127-c)
        A = wT_kq.reshape(16, 2, 128, 8, 128)[:, :, :, :, ::-1]
        wT8 = (A.transpose(2, 3, 0, 4, 1).reshape(128, 128, 256)
               * SCALEQK)
        wTv = np.concatenate(v_parts, axis=0).T                # [H, JCC]
        in_maps.append({
            "hsT16": hsT16,
            "hsT8": hsT8,
            "wT8": np.ascontiguousarray(wT8).astype(ml_dtypes.float8_e4m3),
            "wTv": np.ascontiguousarray(wTv).astype(ml_dtypes.bfloat16),
            "woT": woT_full,
            "cosf": cosf,
            "sinm": sinm,
            "masks": masks16,
        })
    return in_maps


def _assemble(results):
    out = np.empty((B, S, H), dtype=np.float32)
    for c in range(NCORES):
        b, q = divmod(c, NCORES // B)
        res = results[c]["out"]                    # [2*QT, H]
        out[b][QT * q:QT * (q + 1)] = res[:QT]
        out[b][TH + QT * q:TH + QT * (q + 1)] = res[QT:]
    return out


def kernel(hidden_states, positions, w_pack, w_o):
    import os
    os.environ["BASS_NEVER_TRACE"] = "1"
    nc = _get_nc()
    in_maps = _host_inputs(hidden_states, positions, w_pack, w_o)
    res = run_bass_kernel_spmd(nc, in_maps, list(range(NCORES)))
    return _assemble(res.results)
